# revision 17
# baseline (speedup 1.0000x reference)
"""Trainium2 Bass kernel for GQA decode attention (B=8, S=1, D=4096, H=32, KVH=8, HD=128, M=4096).

Sharding: KV heads across 8 cores (tensor parallel). Per core: 1 KV head, 4 Q heads,
wq/wk/wv column-parallel, wo row-parallel (host sums partials), KV cache sharded on head axis.

Trick: the decode position `pos` is read host-side and each core's cache slice is rolled so
the updated row lands at rolled-row 0. Softmax/attention are invariant to key order, so the
device program is fully static (one compile serves every invocation).

Device per-core layout notes:
 - cache tiles in SBUF as [128, 2048] halves via "(p n) d -> p (n d)": partition p holds rows
   p*32+n -> 8KB contiguous per partition per DMA.  Chunk n (free cols n*128..n*128+128)
   is a [128, 128] tile whose partition j corresponds to cache row m = j*32 + n.
 - scores kept as two 4-batch "waves"; wave-local batch l occupies partitions 32l..32l+4
   (matmul PSUM outputs must start at 32-aligned partitions).
 - q^T layout [128, 32] with column h*8+b (from 4 per-head PE transposes of [8,128]).
"""

import sys
import math

if "/opt/trn_rl_repo" not in sys.path:
    sys.path.insert(0, "/opt/trn_rl_repo")

import numpy as np

B, S, D = 8, 1, 4096
H, KVH, HD = 32, 8, 128
M = 4096
NCORES = 8
QH = H // NCORES          # 4 query heads per core
SCALE = 1.0 / math.sqrt(HD)
NT = M // 128             # 32 chunks of the cache
KT = D // 128             # 32 contraction tiles for projections

_BUILT = {}


def _build_nc():
    import concourse.bass as bass
    import concourse.mybir as mybir
    from concourse import bacc, tile

    f32 = mybir.dt.float32
    AF = mybir.ActivationFunctionType
    ALU = mybir.AluOpType
    AX = mybir.AxisListType

    nc = bacc.Bacc(None)

    # ---- per-core inputs ----
    xT4_d = nc.dram_tensor("xT4", [128, KT, B, 4], f32, kind="ExternalInput")
    wq_d = nc.dram_tensor("wq", [128, KT, QH * HD], f32, kind="ExternalInput")
    wk_d = nc.dram_tensor("wk", [128, KT, HD], f32, kind="ExternalInput")
    wv_d = nc.dram_tensor("wv", [128, KT, HD], f32, kind="ExternalInput")
    # wo regrouped host-side as [128, nch, h, 512] (partition-contiguous)
    wo_d = nc.dram_tensor("wo", [128, 8, QH, 512], f32, kind="ExternalInput")
    ck_d = nc.dram_tensor("ckT", [B, 128, M], f32, kind="ExternalInput")
    cv_d = nc.dram_tensor("cv", [B, M, HD], f32, kind="ExternalInput")
    cosq_d = nc.dram_tensor("cosq", [B, QH * HD], f32, kind="ExternalInput")
    sinq_d = nc.dram_tensor("sinq", [B, QH * HD], f32, kind="ExternalInput")
    cosk_d = nc.dram_tensor("cosk", [B, HD], f32, kind="ExternalInput")
    sink_d = nc.dram_tensor("sink", [B, HD], f32, kind="ExternalInput")
    id_d = nc.dram_tensor("id128", [128, 128], f32, kind="ExternalInput")

    # ---- per-core outputs ----
    out0_d = nc.dram_tensor("out0", [4, D], f32, kind="ExternalOutput")
    out1_d = nc.dram_tensor("out1", [4, D], f32, kind="ExternalOutput")
    knew_d = nc.dram_tensor("knew", [B, HD], f32, kind="ExternalOutput")
    vnew_d = nc.dram_tensor("vnew", [B, HD], f32, kind="ExternalOutput")

    HM = M // 2  # half of the cache free dim (2048)

    with tile.TileContext(nc) as tc:
        with (
            tc.tile_pool(name="const", bufs=1) as constp,
            tc.tile_pool(name="wq", bufs=2) as wqp,
            tc.tile_pool(name="wkv", bufs=1) as wkvp,
            tc.tile_pool(name="wo", bufs=3) as wop,
            tc.tile_pool(name="kv", bufs=11) as kvp,
            tc.tile_pool(name="sc", bufs=2) as scp,
            tc.tile_pool(name="sm", bufs=2) as smp,
            tc.tile_pool(name="pt", bufs=2) as ptp,
            tc.tile_pool(name="tail", bufs=3) as tailp,
        ):
            # ---------- constants ----------
            id_sb = constp.tile([128, 128], f32, tag="id")
            nc.sync.dma_start(id_sb[:], id_d[:])
            xT4_sb = constp.tile([128, KT, B, 4], f32, tag="xT4")
            nc.sync.dma_start(xT4_sb[:], xT4_d[:])
            cosq_sb = constp.tile([B, QH * HD], f32, tag="cosq")
            nc.sync.dma_start(cosq_sb[:], cosq_d[:])
            sinq_sb = constp.tile([B, QH * HD], f32, tag="sinq")
            nc.sync.dma_start(sinq_sb[:], sinq_d[:])
            cosk_sb = constp.tile([B, HD], f32, tag="cosk")
            nc.sync.dma_start(cosk_sb[:], cosk_d[:])
            sink_sb = constp.tile([B, HD], f32, tag="sink")
            nc.sync.dma_start(sink_sb[:], sink_d[:])

            # ---------- HAM warm-up: keep PE busy during startup DMA ----------
            with tc.tile_pool(name="pswarm", bufs=1, space="PSUM") as pswarm:
                warm_ps = pswarm.tile([128, 512], f32, tag="warm")
                for _ in range(24):
                    nc.tensor.matmul(warm_ps[:, 0:128], id_sb[:], id_sb[:])

            # ---------- projections q/k/v ----------
            q_sb = constp.tile([B, QH * HD], f32, tag="qsb")
            k_sb = constp.tile([B, HD], f32, tag="ksb")
            v_sb = constp.tile([B, HD], f32, tag="vsb")
            qT_sb = constp.tile([128, H], f32, tag="qTsb")

            with tc.tile_pool(name="psproj", bufs=1, space="PSUM") as psproj:
                q_ps = psproj.tile([B, QH * HD], f32, tag="qps")
                k_ps = psproj.tile([B, HD], f32, tag="kps")
                v_ps = psproj.tile([B, HD], f32, tag="vps")

                for half in range(2):
                    wk_sb = wkvp.tile([128, 16, HD], f32, tag="wk")
                    nc.sync.dma_start(
                        wk_sb[:], wk_d[:, 16 * half : 16 * (half + 1), :]
                    )
                    wv_sb = wkvp.tile([128, 16, HD], f32, tag="wv")
                    nc.sync.dma_start(
                        wv_sb[:], wv_d[:, 16 * half : 16 * (half + 1), :]
                    )
                    for ch in range(4):
                        wq_sb = wqp.tile([128, 4, QH * HD], f32, tag="wqc")
                        cq = half * 4 + ch
                        nc.sync.dma_start(
                            wq_sb[:], wq_d[:, cq * 4 : (cq + 1) * 4, :]
                        )
                        for i in range(4):
                            n = cq * 4 + i
                            ni = n - 16 * half
                            lhsT = xT4_sb[:, n, :, 0]  # [128, 8] x^T tile
                            nc.tensor.matmul(
                                q_ps[:], lhsT, wq_sb[:, i, :],
                                start=(n == 0), stop=(n == KT - 1),
                            )
                            nc.tensor.matmul(
                                k_ps[:], lhsT, wk_sb[:, ni, :],
                                start=(n == 0), stop=(n == KT - 1),
                            )
                            nc.tensor.matmul(
                                v_ps[:], lhsT, wv_sb[:, ni, :],
                                start=(n == 0), stop=(n == KT - 1),
                            )

                # ---- RoPE (scale folded into cosq/sinq for q) ----
                qr_sb = constp.tile([B, QH * HD], f32, tag="qrot")
                qv = q_ps.rearrange("b (t two) -> b t two", two=2)
                qrv = qr_sb.rearrange("b (t two) -> b t two", two=2)
                nc.vector.tensor_scalar_mul(qrv[:, :, 0], qv[:, :, 1], -1.0)
                nc.vector.tensor_copy(qrv[:, :, 1], qv[:, :, 0])
                nc.vector.tensor_tensor(q_sb[:], q_ps[:], cosq_sb[:], ALU.mult)
                nc.vector.tensor_tensor(qr_sb[:], qr_sb[:], sinq_sb[:], ALU.mult)
                nc.vector.tensor_tensor(q_sb[:], q_sb[:], qr_sb[:], ALU.add)

                kr_sb = constp.tile([B, HD], f32, tag="krot")
                kv_ = k_ps.rearrange("b (t two) -> b t two", two=2)
                krv = kr_sb.rearrange("b (t two) -> b t two", two=2)
                nc.vector.tensor_scalar_mul(krv[:, :, 0], kv_[:, :, 1], -1.0)
                nc.vector.tensor_copy(krv[:, :, 1], kv_[:, :, 0])
                nc.vector.tensor_tensor(k_sb[:], k_ps[:], cosk_sb[:], ALU.mult)
                nc.vector.tensor_tensor(kr_sb[:], kr_sb[:], sink_sb[:], ALU.mult)
                nc.vector.tensor_tensor(k_sb[:], k_sb[:], kr_sb[:], ALU.add)

                nc.scalar.copy(v_sb[:], v_ps[:])

                nc.sync.dma_start(knew_d[:], k_sb[:])
                nc.sync.dma_start(vnew_d[:], v_sb[:])

                # ---- q^T: [128, 32], column h*8+b ----
                qT_ps = psproj.tile([128, H], f32, tag="qT")
                for h in range(QH):
                    nc.tensor.transpose(
                        qT_ps[:, h * B : (h + 1) * B],
                        q_sb[:, h * HD : (h + 1) * HD],
                        id_sb[0:B, 0:B],
                    )
                nc.scalar.copy(qT_sb[:], qT_ps[:])
            qTv = qT_sb.rearrange("p (h b) -> p h b", b=B)

            # ---------- attention: two waves of 4 batches ----------
            aoT_all = []
            with (
                tc.tile_pool(name="pstp", bufs=2, space="PSUM") as pstp,
                tc.tile_pool(name="pss", bufs=2, space="PSUM") as pss,
                tc.tile_pool(name="psv", bufs=4, space="PSUM") as psv,
            ):
                # k_new^T for the rolled-row-0 column overwrite of K^T tiles
                kT_ps = pstp.tile([128, B], f32, tag="tp")
                nc.tensor.transpose(kT_ps[:, 0:B], k_sb[:], id_sb[0:B, 0:B])
                kTn_sb = constp.tile([128, B], f32, tag="kTn")
                nc.scalar.copy(kTn_sb[:], kT_ps[:, 0:B])
                for w in range(2):
                    Kh = []  # Kh[l][half] : K^T halves [128 (d), 2048 (m-permuted)]
                    for l in range(4):
                        b = 4 * w + l
                        halves = []
                        for hf in range(2):
                            t = kvp.tile([128, HM], f32, tag="kv")
                            nc.scalar.dma_start(
                                t[:], ck_d[b][:, hf * HM : (hf + 1) * HM]
                            )
                            if hf == 0:
                                nc.vector.tensor_copy(t[:, 0:1], kTn_sb[:, b : b + 1])
                            halves.append(t)
                        Kh.append(halves)

                    s_sb = scp.tile([128, M], f32, tag="s")
                    den8 = smp.tile([128, 8], f32, tag="den8")
                    for g in range(8):
                        s_ps = pss.tile([128, 512], f32, tag="sps")
                        nc.vector.memset(s_ps[:], 0.0)
                        hf, gf = divmod(g, 4)
                        for l in range(4):
                            b = 4 * w + l
                            nc.tensor.matmul(
                                s_ps[32 * l : 32 * l + QH, :],
                                qTv[:, :, b],
                                Kh[l][hf][:, gf * 512 : (gf + 1) * 512],
                                tile_position=(0, 32 * l),
                            )
                        # exp straight out of PSUM (bounded scores: no max-sub; zero mask)
                        nc.scalar.activation(
                            s_sb[:, g * 512 : (g + 1) * 512],
                            s_ps[:],
                            AF.Exp, scale=1.0,
                            accum_out=den8[:, g : g + 1],
                        )

                    den = smp.tile([128, 1], f32, tag="den")
                    nc.vector.tensor_reduce(den[:], den8[:], AX.X, ALU.add)
                    inv = smp.tile([128, 1], f32, tag="inv")
                    nc.vector.reciprocal(inv[:], den[:])

                    Vh = []
                    for l in range(4):
                        b = 4 * w + l
                        src = cv_d[b].rearrange("(p n) d -> p (n d)", p=128)
                        halves = []
                        for hf in range(2):
                            t = kvp.tile([128, HM], f32, tag="kv")
                            nc.scalar.dma_start(t[:], src[:, hf * HM : (hf + 1) * HM])
                            if hf == 0:
                                nc.sync.dma_start(t[0:1, 0:HD], v_sb[b : b + 1, :])
                            halves.append(t)
                        Vh.append(halves)

                    # ---- PV (one PSUM bank per wave-batch: whole-bank has_written) ----
                    pv_ps = []
                    for l in range(4):
                        pvt = psv.tile([128, HD], f32, tag="pv")
                        pv_ps.append(pvt)
                    for n in range(NT):
                        hf, nf = divmod(n, 16)
                        pT_ps = pstp.tile([128, 128], f32, tag="tp")
                        nc.tensor.transpose(
                            pT_ps[:], s_sb[:, n * 128 : (n + 1) * 128], id_sb[:]
                        )
                        pT_sb = ptp.tile([128, 128], f32, tag="pTsb")
                        nc.scalar.copy(pT_sb[:], pT_ps[:])
                        for l in range(4):
                            nc.tensor.matmul(
                                pv_ps[l][32 * l : 32 * l + QH, :],
                                pT_sb[:, 32 * l : 32 * l + QH],
                                Vh[l][hf][:, nf * 128 : (nf + 1) * 128],
                                start=(n == 0),
                                stop=(n == NT - 1),
                                tile_position=(0, 32 * l),
                            )

                    att_sb = smp.tile([128, HD], f32, tag="att")
                    nc.gpsimd.memset(att_sb[:], 0.0)
                    for l in range(4):
                        nc.vector.tensor_scalar_mul(
                            att_sb[32 * l : 32 * l + QH, :],
                            pv_ps[l][32 * l : 32 * l + QH, :],
                            inv[32 * l : 32 * l + QH, :],
                        )

                    aoT_ps = psv.tile([128, 128], f32, tag="pv")
                    nc.tensor.transpose(aoT_ps[:], att_sb[:], id_sb[:])
                    aoT_sb = constp.tile([128, 128], f32, tag=f"aoT{w}")
                    nc.scalar.copy(aoT_sb[:], aoT_ps[:])
                    aoT_all.append(aoT_sb.rearrange("p (l c) -> p l c", l=4))

                # ---------- output projection (both waves share wo stream) ----------
                for nch in range(8):
                    wo_sb = wop.tile([128, QH, 512], f32, tag="wo")
                    nc.sync.dma_start(wo_sb[:], wo_d[:, nch, :, :])
                    for w in range(2):
                        wo_ps = psv.tile([4, 512], f32, tag="pv")
                        for h in range(QH):
                            nc.tensor.matmul(
                                wo_ps[:],
                                aoT_all[w][:, :, h],
                                wo_sb[:, h, :],
                                start=(h == 0),
                                stop=(h == QH - 1),
                            )
                        ob = tailp.tile([4, 512], f32, tag="osb")
                        nc.vector.tensor_copy(ob[:], wo_ps[:])
                        nc.sync.dma_start(
                            (out0_d if w == 0 else out1_d)[:, nch * 512 : (nch + 1) * 512],
                            ob[:],
                        )

    nc.compile()
    return nc


def _get_nc():
    if "nc" not in _BUILT:
        _BUILT["nc"] = _build_nc()
    return _BUILT["nc"]


def _rope_tables(freqs_cos, freqs_sin):
    """Interleaved full-width rope tables: c[2t]=c[2t+1]=cos[t]."""
    c = np.empty(HD, np.float32)
    s = np.empty(HD, np.float32)
    c[0::2] = c[1::2] = freqs_cos[0]
    s[0::2] = s[1::2] = freqs_sin[0]
    return c, s


def _reference_fallback(x, freqs_cos, freqs_sin, mask, cache_k, cache_v, prefill,
                        input_indexes, cache_indexes, wq, wk, wv, wo):
    """Plain numpy decode/prefill reference (used only for prefill != 0)."""
    bsz = x.shape[0]
    n_rep = H // KVH
    xq = (x.reshape(bsz, D) @ wq).reshape(bsz, 1, H, HD)
    xk = (x.reshape(bsz, D) @ wk).reshape(bsz, 1, KVH, HD)
    xv = (x.reshape(bsz, D) @ wv).reshape(bsz, 1, KVH, HD)

    def rope(t):
        r, i = t[..., 0::2], t[..., 1::2]
        c = freqs_cos[None, :, None, :]
        s = freqs_sin[None, :, None, :]
        out = np.empty_like(t)
        out[..., 0::2] = r * c - i * s
        out[..., 1::2] = r * s + i * c
        return out

    xq = rope(xq)
    xk = rope(xk)
    xk = np.swapaxes(xk, 1, 2)
    xv = np.swapaxes(xv, 1, 2)
    if prefill:
        ck, cvv = xk, xv
    else:
        pos = int(np.asarray(input_indexes).reshape(-1)[0])
        ck = cache_k.copy()
        cvv = cache_v.copy()
        ck[:, :, pos, :] = xk[:, :, 0, :]
        cvv[:, :, pos, :] = xv[:, :, 0, :]
    keys = np.repeat(ck, n_rep, axis=1)
    values = np.repeat(cvv, n_rep, axis=1)
    scores = np.einsum("bshd,bhmd->bhsm", xq.astype(np.float32), keys) / math.sqrt(HD)
    scores = scores + mask
    scores = scores - scores.max(-1, keepdims=True)
    p = np.exp(scores)
    p /= p.sum(-1, keepdims=True)
    out = np.einsum("bhsm,bhmd->bhsd", p, values)
    out = np.swapaxes(out, 1, 2).reshape(bsz, 1, H * HD)
    return (out @ wo).astype(np.float32), ck, cvv


def make_in_maps(x, freqs_cos, freqs_sin, mask, cache_k, cache_v, pos, wq, wk, wv, wo):
    """Build the 8 per-core input dicts (host-side sharding)."""
    col = np.arange(M)
    perm = ((col % 128) * 32 + col // 128 + pos) % M

    cos1, sin1 = _rope_tables(freqs_cos, freqs_sin)
    cosq = np.broadcast_to(np.tile(cos1, QH) * SCALE, (B, QH * HD)).copy()
    sinq = np.broadcast_to(np.tile(sin1, QH) * SCALE, (B, QH * HD)).copy()
    cosk = np.broadcast_to(cos1, (B, HD)).copy()
    sink = np.broadcast_to(sin1, (B, HD)).copy()
    id128 = np.eye(128, dtype=np.float32)

    xx = x.reshape(B, KT, 128).transpose(2, 1, 0)        # [128, KT, B]
    xT4 = np.ascontiguousarray(
        np.repeat(xx[:, :, :, None], 4, axis=3).astype(np.float32))  # [128,KT,B,4]

    in_maps = []
    for c in range(NCORES):
        qh0 = QH * c
        wo_c = wo[qh0 * HD : (qh0 + QH) * HD]            # [512, 4096]
        wo_r = np.ascontiguousarray(
            wo_c.reshape(QH, 128, 8, 512).transpose(1, 2, 0, 3))  # [128,nch,h,512]
        in_maps.append({
            "xT4": xT4,
            "wq": np.ascontiguousarray(
                wq[:, qh0 * HD : (qh0 + QH) * HD].reshape(KT, 128, QH * HD)
                .transpose(1, 0, 2)),
            "wk": np.ascontiguousarray(
                wk[:, c * HD : (c + 1) * HD].reshape(KT, 128, HD)
                .transpose(1, 0, 2)),
            "wv": np.ascontiguousarray(
                wv[:, c * HD : (c + 1) * HD].reshape(KT, 128, HD)
                .transpose(1, 0, 2)),
            "wo": wo_r,
            "ckT": np.ascontiguousarray(
                cache_k[:, c][:, perm, :].transpose(0, 2, 1)),
            "cv": np.ascontiguousarray(np.roll(cache_v[:, c], -pos, axis=1)),
            "cosq": cosq, "sinq": sinq, "cosk": cosk, "sink": sink,
            "id128": id128,
        })
    return in_maps


def kernel(x, freqs_cos, freqs_sin, mask, cache_k, cache_v, prefill,
           input_indexes, cache_indexes, wq, wk, wv, wo):
    x = np.asarray(x, np.float32)
    freqs_cos = np.asarray(freqs_cos, np.float32)
    freqs_sin = np.asarray(freqs_sin, np.float32)
    mask = np.asarray(mask, np.float32)
    cache_k = np.asarray(cache_k, np.float32)
    cache_v = np.asarray(cache_v, np.float32)
    wq = np.asarray(wq, np.float32)
    wk = np.asarray(wk, np.float32)
    wv = np.asarray(wv, np.float32)
    wo = np.asarray(wo, np.float32)

    pre = np.asarray(prefill)
    if int(pre.reshape(-1)[0]) if pre.size else 0:
        return _reference_fallback(x, freqs_cos, freqs_sin, mask, cache_k, cache_v,
                                   1, input_indexes, cache_indexes, wq, wk, wv, wo)

    if np.any(mask):
        return _reference_fallback(x, freqs_cos, freqs_sin, mask, cache_k, cache_v,
                                   0, input_indexes, cache_indexes, wq, wk, wv, wo)

    pos = int(np.asarray(input_indexes).reshape(-1)[0])

    from concourse.bass_utils import run_bass_kernel_spmd

    nc = _get_nc()
    in_maps = make_in_maps(x, freqs_cos, freqs_sin, mask, cache_k, cache_v, pos,
                           wq, wk, wv, wo)

    res = run_bass_kernel_spmd(nc, in_maps, list(range(NCORES))).results

    out = np.zeros((B, D), np.float32)
    knew = np.empty((B, KVH, HD), np.float32)
    vnew = np.empty((B, KVH, HD), np.float32)
    for c in range(NCORES):
        out[0:4] += res[c]["out0"]
        out[4:8] += res[c]["out1"]
        knew[:, c] = res[c]["knew"]
        vnew[:, c] = res[c]["vnew"]

    ck_out = cache_k.copy()
    cv_out = cache_v.copy()
    ck_out[:, :, pos, :] = knew
    cv_out[:, :, pos, :] = vnew
    return out.reshape(B, S, H * HD), ck_out, cv_out


# revision 18
# speedup vs baseline: 1.1406x; 1.1406x over previous
"""Trainium2 Bass kernel for GQA decode attention (B=8, S=1, D=4096, H=32, KVH=8, HD=128, M=4096).

Sharding: KV heads across 8 cores (tensor parallel). Per core: 1 KV head, 4 Q heads,
wq/wk/wv column-parallel, wo row-parallel (host sums partials), KV cache sharded on head axis.

Trick: the decode position `pos` is read host-side and each core's cache slice is rolled so
the updated row lands at rolled-row 0. Softmax/attention are invariant to key order, so the
device program is fully static (one compile serves every invocation).

Device per-core layout notes:
 - cache tiles in SBUF as [128, 2048] halves via "(p n) d -> p (n d)": partition p holds rows
   p*32+n -> 8KB contiguous per partition per DMA.  Chunk n (free cols n*128..n*128+128)
   is a [128, 128] tile whose partition j corresponds to cache row m = j*32 + n.
 - scores kept as two 4-batch "waves"; wave-local batch l occupies partitions 32l..32l+4
   (matmul PSUM outputs must start at 32-aligned partitions).
 - q^T layout [128, 32] with column h*8+b (from 4 per-head PE transposes of [8,128]).
"""

import sys
import math

if "/opt/trn_rl_repo" not in sys.path:
    sys.path.insert(0, "/opt/trn_rl_repo")

import numpy as np

B, S, D = 8, 1, 4096
H, KVH, HD = 32, 8, 128
M = 4096
NCORES = 8
QH = H // NCORES          # 4 query heads per core
SCALE = 1.0 / math.sqrt(HD)
NT = M // 128             # 32 chunks of the cache
KT = D // 128             # 32 contraction tiles for projections

_BUILT = {}


def _build_nc():
    import concourse.bass as bass
    import concourse.mybir as mybir
    from concourse import bacc, tile

    f32 = mybir.dt.float32
    AF = mybir.ActivationFunctionType
    ALU = mybir.AluOpType
    AX = mybir.AxisListType

    nc = bacc.Bacc(None)

    # ---- per-core inputs ----
    xT4_d = nc.dram_tensor("xT4", [128, KT, B, 4], f32, kind="ExternalInput")
    wq_d = nc.dram_tensor("wq", [128, KT, QH * HD], f32, kind="ExternalInput")
    wk_d = nc.dram_tensor("wk", [128, KT, HD], f32, kind="ExternalInput")
    wv_d = nc.dram_tensor("wv", [128, KT, HD], f32, kind="ExternalInput")
    # wo regrouped host-side as [128, nch, h, 512] (partition-contiguous)
    wo_d = nc.dram_tensor("wo", [128, 8, QH, 512], f32, kind="ExternalInput")
    ck_d = nc.dram_tensor("ckT", [B, 128, M], f32, kind="ExternalInput")
    cv_d = nc.dram_tensor("cv", [B, M, HD], f32, kind="ExternalInput")
    cosq_d = nc.dram_tensor("cosq", [B, QH * HD], f32, kind="ExternalInput")
    sinq_d = nc.dram_tensor("sinq", [B, QH * HD], f32, kind="ExternalInput")
    cosk_d = nc.dram_tensor("cosk", [B, HD], f32, kind="ExternalInput")
    sink_d = nc.dram_tensor("sink", [B, HD], f32, kind="ExternalInput")
    id_d = nc.dram_tensor("id128", [128, 128], f32, kind="ExternalInput")

    # ---- per-core outputs ----
    out0_d = nc.dram_tensor("out0", [4, D], f32, kind="ExternalOutput")
    out1_d = nc.dram_tensor("out1", [4, D], f32, kind="ExternalOutput")
    knew_d = nc.dram_tensor("knew", [B, HD], f32, kind="ExternalOutput")
    vnew_d = nc.dram_tensor("vnew", [B, HD], f32, kind="ExternalOutput")

    HM = M // 2  # half of the cache free dim (2048)

    with tile.TileContext(nc) as tc:
        with (
            tc.tile_pool(name="const", bufs=1) as constp,
            tc.tile_pool(name="wq", bufs=2) as wqp,
            tc.tile_pool(name="wkv", bufs=1) as wkvp,
            tc.tile_pool(name="wo", bufs=3) as wop,
            tc.tile_pool(name="kv", bufs=11) as kvp,
            tc.tile_pool(name="sc", bufs=2) as scp,
            tc.tile_pool(name="sm", bufs=2) as smp,
            tc.tile_pool(name="pt", bufs=2) as ptp,
            tc.tile_pool(name="tail", bufs=3) as tailp,
        ):
            # ---------- constants ----------
            id_sb = constp.tile([128, 128], f32, tag="id")
            nc.sync.dma_start(id_sb[:], id_d[:])
            xT4_sb = constp.tile([128, KT, B, 4], f32, tag="xT4")
            nc.sync.dma_start(xT4_sb[:], xT4_d[:])
            cosq_sb = constp.tile([B, QH * HD], f32, tag="cosq")
            nc.sync.dma_start(cosq_sb[:], cosq_d[:])
            sinq_sb = constp.tile([B, QH * HD], f32, tag="sinq")
            nc.sync.dma_start(sinq_sb[:], sinq_d[:])
            cosk_sb = constp.tile([B, HD], f32, tag="cosk")
            nc.sync.dma_start(cosk_sb[:], cosk_d[:])
            sink_sb = constp.tile([B, HD], f32, tag="sink")
            nc.sync.dma_start(sink_sb[:], sink_d[:])

            # ---------- HAM warm-up: keep PE busy during startup DMA ----------
            with tc.tile_pool(name="pswarm", bufs=1, space="PSUM") as pswarm:
                warm_ps = pswarm.tile([128, 512], f32, tag="warm")
                for _ in range(24):
                    nc.tensor.matmul(warm_ps[:, 0:128], id_sb[:], id_sb[:])

            # ---------- projections q/k/v ----------
            q_sb = constp.tile([B, QH * HD], f32, tag="qsb")
            k_sb = constp.tile([B, HD], f32, tag="ksb")
            v_sb = constp.tile([B, HD], f32, tag="vsb")
            qT_sb = constp.tile([128, H], f32, tag="qTsb")

            with tc.tile_pool(name="psproj", bufs=1, space="PSUM") as psproj:
                q_ps = psproj.tile([B, QH * HD], f32, tag="qps")
                k_ps = psproj.tile([B, HD], f32, tag="kps")
                v_ps = psproj.tile([B, HD], f32, tag="vps")

                for half in range(2):
                    wk_sb = wkvp.tile([128, 16, HD], f32, tag="wk")
                    nc.sync.dma_start(
                        wk_sb[:], wk_d[:, 16 * half : 16 * (half + 1), :]
                    )
                    wv_sb = wkvp.tile([128, 16, HD], f32, tag="wv")
                    nc.sync.dma_start(
                        wv_sb[:], wv_d[:, 16 * half : 16 * (half + 1), :]
                    )
                    for ch in range(4):
                        wq_sb = wqp.tile([128, 4, QH * HD], f32, tag="wqc")
                        cq = half * 4 + ch
                        nc.sync.dma_start(
                            wq_sb[:], wq_d[:, cq * 4 : (cq + 1) * 4, :]
                        )
                        for i in range(4):
                            n = cq * 4 + i
                            ni = n - 16 * half
                            lhsT = xT4_sb[:, n, :, 0]  # [128, 8] x^T tile
                            nc.tensor.matmul(
                                q_ps[:], lhsT, wq_sb[:, i, :],
                                start=(n == 0), stop=(n == KT - 1),
                            )
                            nc.tensor.matmul(
                                k_ps[:], lhsT, wk_sb[:, ni, :],
                                start=(n == 0), stop=(n == KT - 1),
                            )
                            nc.tensor.matmul(
                                v_ps[:], lhsT, wv_sb[:, ni, :],
                                start=(n == 0), stop=(n == KT - 1),
                            )

                # ---- RoPE (scale folded into cosq/sinq for q) ----
                qr_sb = constp.tile([B, QH * HD], f32, tag="qrot")
                qv = q_ps.rearrange("b (t two) -> b t two", two=2)
                qrv = qr_sb.rearrange("b (t two) -> b t two", two=2)
                nc.vector.tensor_scalar_mul(qrv[:, :, 0], qv[:, :, 1], -1.0)
                nc.vector.tensor_copy(qrv[:, :, 1], qv[:, :, 0])
                nc.vector.tensor_tensor(q_sb[:], q_ps[:], cosq_sb[:], ALU.mult)
                nc.vector.tensor_tensor(qr_sb[:], qr_sb[:], sinq_sb[:], ALU.mult)
                nc.vector.tensor_tensor(q_sb[:], q_sb[:], qr_sb[:], ALU.add)

                kr_sb = constp.tile([B, HD], f32, tag="krot")
                kv_ = k_ps.rearrange("b (t two) -> b t two", two=2)
                krv = kr_sb.rearrange("b (t two) -> b t two", two=2)
                nc.vector.tensor_scalar_mul(krv[:, :, 0], kv_[:, :, 1], -1.0)
                nc.vector.tensor_copy(krv[:, :, 1], kv_[:, :, 0])
                nc.vector.tensor_tensor(k_sb[:], k_ps[:], cosk_sb[:], ALU.mult)
                nc.vector.tensor_tensor(kr_sb[:], kr_sb[:], sink_sb[:], ALU.mult)
                nc.vector.tensor_tensor(k_sb[:], k_sb[:], kr_sb[:], ALU.add)

                nc.scalar.copy(v_sb[:], v_ps[:])

                nc.sync.dma_start(knew_d[:], k_sb[:])
                nc.sync.dma_start(vnew_d[:], v_sb[:])

                # ---- q^T: [128, 32], column h*8+b ----
                qT_ps = psproj.tile([128, H], f32, tag="qT")
                for h in range(QH):
                    nc.tensor.transpose(
                        qT_ps[:, h * B : (h + 1) * B],
                        q_sb[:, h * HD : (h + 1) * HD],
                        id_sb[0:B, 0:B],
                    )
                nc.scalar.copy(qT_sb[:], qT_ps[:])
            qTv = qT_sb.rearrange("p (h b) -> p h b", b=B)

            # ---------- attention: two waves of 4 batches ----------
            aoT_all = []
            with (
                tc.tile_pool(name="pstp", bufs=2, space="PSUM") as pstp,
                tc.tile_pool(name="pss", bufs=2, space="PSUM") as pss,
                tc.tile_pool(name="psv", bufs=4, space="PSUM") as psv,
            ):
                # k_new^T for the rolled-row-0 column overwrite of K^T tiles
                kT_ps = pstp.tile([128, B], f32, tag="tp")
                nc.tensor.transpose(kT_ps[:, 0:B], k_sb[:], id_sb[0:B, 0:B])
                kTn_sb = constp.tile([128, B], f32, tag="kTn")
                nc.scalar.copy(kTn_sb[:], kT_ps[:, 0:B])
                for w in range(2):
                    Kh = []  # Kh[l][half] : K^T halves [128 (d), 2048 (m-permuted)]
                    for l in range(4):
                        b = 4 * w + l
                        halves = []
                        for hf in range(2):
                            t = kvp.tile([128, HM], f32, tag="kv")
                            nc.sync.dma_start(
                                t[:], ck_d[b][:, hf * HM : (hf + 1) * HM]
                            )
                            if hf == 0:
                                nc.vector.tensor_copy(t[:, 0:1], kTn_sb[:, b : b + 1])
                            halves.append(t)
                        Kh.append(halves)

                    s_sb = scp.tile([128, M], f32, tag="s")
                    den8 = smp.tile([128, 8], f32, tag="den8")
                    for g in range(8):
                        s_ps = pss.tile([128, 512], f32, tag="sps")
                        nc.vector.memset(s_ps[:], 0.0)
                        hf, gf = divmod(g, 4)
                        for l in range(4):
                            b = 4 * w + l
                            nc.tensor.matmul(
                                s_ps[32 * l : 32 * l + QH, :],
                                qTv[:, :, b],
                                Kh[l][hf][:, gf * 512 : (gf + 1) * 512],
                                tile_position=(0, 32 * l),
                            )
                        # exp straight out of PSUM (bounded scores: no max-sub; zero mask)
                        nc.scalar.activation(
                            s_sb[:, g * 512 : (g + 1) * 512],
                            s_ps[:],
                            AF.Exp, scale=1.0,
                            accum_out=den8[:, g : g + 1],
                        )

                    den = smp.tile([128, 1], f32, tag="den")
                    nc.vector.tensor_reduce(den[:], den8[:], AX.X, ALU.add)
                    inv = smp.tile([128, 1], f32, tag="inv")
                    nc.vector.reciprocal(inv[:], den[:])

                    Vh = []
                    for l in range(4):
                        b = 4 * w + l
                        src = cv_d[b].rearrange("(p n) d -> p (n d)", p=128)
                        halves = []
                        for hf in range(2):
                            t = kvp.tile([128, HM], f32, tag="kv")
                            nc.sync.dma_start(t[:], src[:, hf * HM : (hf + 1) * HM])
                            if hf == 0:
                                nc.sync.dma_start(t[0:1, 0:HD], v_sb[b : b + 1, :])
                            halves.append(t)
                        Vh.append(halves)

                    # ---- PV (one PSUM bank per wave-batch: whole-bank has_written) ----
                    pv_ps = []
                    for l in range(4):
                        pvt = psv.tile([128, HD], f32, tag="pv")
                        pv_ps.append(pvt)
                    for n in range(NT):
                        hf, nf = divmod(n, 16)
                        pT_ps = pstp.tile([128, 128], f32, tag="tp")
                        nc.tensor.transpose(
                            pT_ps[:], s_sb[:, n * 128 : (n + 1) * 128], id_sb[:]
                        )
                        pT_sb = ptp.tile([128, 128], f32, tag="pTsb")
                        nc.scalar.copy(pT_sb[:], pT_ps[:])
                        for l in range(4):
                            nc.tensor.matmul(
                                pv_ps[l][32 * l : 32 * l + QH, :],
                                pT_sb[:, 32 * l : 32 * l + QH],
                                Vh[l][hf][:, nf * 128 : (nf + 1) * 128],
                                start=(n == 0),
                                stop=(n == NT - 1),
                                tile_position=(0, 32 * l),
                            )

                    att_sb = smp.tile([128, HD], f32, tag="att")
                    nc.gpsimd.memset(att_sb[:], 0.0)
                    for l in range(4):
                        nc.vector.tensor_scalar_mul(
                            att_sb[32 * l : 32 * l + QH, :],
                            pv_ps[l][32 * l : 32 * l + QH, :],
                            inv[32 * l : 32 * l + QH, :],
                        )

                    aoT_ps = psv.tile([128, 128], f32, tag="pv")
                    nc.tensor.transpose(aoT_ps[:], att_sb[:], id_sb[:])
                    aoT_sb = constp.tile([128, 128], f32, tag=f"aoT{w}")
                    nc.scalar.copy(aoT_sb[:], aoT_ps[:])
                    aoT_all.append(aoT_sb.rearrange("p (l c) -> p l c", l=4))

                # ---------- output projection (both waves share wo stream) ----------
                for nch in range(8):
                    wo_sb = wop.tile([128, QH, 512], f32, tag="wo")
                    nc.sync.dma_start(wo_sb[:], wo_d[:, nch, :, :])
                    for w in range(2):
                        wo_ps = psv.tile([4, 512], f32, tag="pv")
                        for h in range(QH):
                            nc.tensor.matmul(
                                wo_ps[:],
                                aoT_all[w][:, :, h],
                                wo_sb[:, h, :],
                                start=(h == 0),
                                stop=(h == QH - 1),
                            )
                        ob = tailp.tile([4, 512], f32, tag="osb")
                        nc.vector.tensor_copy(ob[:], wo_ps[:])
                        nc.sync.dma_start(
                            (out0_d if w == 0 else out1_d)[:, nch * 512 : (nch + 1) * 512],
                            ob[:],
                        )

    nc.compile()
    return nc


def _get_nc():
    if "nc" not in _BUILT:
        _BUILT["nc"] = _build_nc()
    return _BUILT["nc"]


def _rope_tables(freqs_cos, freqs_sin):
    """Interleaved full-width rope tables: c[2t]=c[2t+1]=cos[t]."""
    c = np.empty(HD, np.float32)
    s = np.empty(HD, np.float32)
    c[0::2] = c[1::2] = freqs_cos[0]
    s[0::2] = s[1::2] = freqs_sin[0]
    return c, s


def _reference_fallback(x, freqs_cos, freqs_sin, mask, cache_k, cache_v, prefill,
                        input_indexes, cache_indexes, wq, wk, wv, wo):
    """Plain numpy decode/prefill reference (used only for prefill != 0)."""
    bsz = x.shape[0]
    n_rep = H // KVH
    xq = (x.reshape(bsz, D) @ wq).reshape(bsz, 1, H, HD)
    xk = (x.reshape(bsz, D) @ wk).reshape(bsz, 1, KVH, HD)
    xv = (x.reshape(bsz, D) @ wv).reshape(bsz, 1, KVH, HD)

    def rope(t):
        r, i = t[..., 0::2], t[..., 1::2]
        c = freqs_cos[None, :, None, :]
        s = freqs_sin[None, :, None, :]
        out = np.empty_like(t)
        out[..., 0::2] = r * c - i * s
        out[..., 1::2] = r * s + i * c
        return out

    xq = rope(xq)
    xk = rope(xk)
    xk = np.swapaxes(xk, 1, 2)
    xv = np.swapaxes(xv, 1, 2)
    if prefill:
        ck, cvv = xk, xv
    else:
        pos = int(np.asarray(input_indexes).reshape(-1)[0])
        ck = cache_k.copy()
        cvv = cache_v.copy()
        ck[:, :, pos, :] = xk[:, :, 0, :]
        cvv[:, :, pos, :] = xv[:, :, 0, :]
    keys = np.repeat(ck, n_rep, axis=1)
    values = np.repeat(cvv, n_rep, axis=1)
    scores = np.einsum("bshd,bhmd->bhsm", xq.astype(np.float32), keys) / math.sqrt(HD)
    scores = scores + mask
    scores = scores - scores.max(-1, keepdims=True)
    p = np.exp(scores)
    p /= p.sum(-1, keepdims=True)
    out = np.einsum("bhsm,bhmd->bhsd", p, values)
    out = np.swapaxes(out, 1, 2).reshape(bsz, 1, H * HD)
    return (out @ wo).astype(np.float32), ck, cvv


def make_in_maps(x, freqs_cos, freqs_sin, mask, cache_k, cache_v, pos, wq, wk, wv, wo):
    """Build the 8 per-core input dicts (host-side sharding)."""
    col = np.arange(M)
    perm = ((col % 128) * 32 + col // 128 + pos) % M

    cos1, sin1 = _rope_tables(freqs_cos, freqs_sin)
    cosq = np.broadcast_to(np.tile(cos1, QH) * SCALE, (B, QH * HD)).copy()
    sinq = np.broadcast_to(np.tile(sin1, QH) * SCALE, (B, QH * HD)).copy()
    cosk = np.broadcast_to(cos1, (B, HD)).copy()
    sink = np.broadcast_to(sin1, (B, HD)).copy()
    id128 = np.eye(128, dtype=np.float32)

    xx = x.reshape(B, KT, 128).transpose(2, 1, 0)        # [128, KT, B]
    xT4 = np.ascontiguousarray(
        np.repeat(xx[:, :, :, None], 4, axis=3).astype(np.float32))  # [128,KT,B,4]

    in_maps = []
    for c in range(NCORES):
        qh0 = QH * c
        wo_c = wo[qh0 * HD : (qh0 + QH) * HD]            # [512, 4096]
        wo_r = np.ascontiguousarray(
            wo_c.reshape(QH, 128, 8, 512).transpose(1, 2, 0, 3))  # [128,nch,h,512]
        in_maps.append({
            "xT4": xT4,
            "wq": np.ascontiguousarray(
                wq[:, qh0 * HD : (qh0 + QH) * HD].reshape(KT, 128, QH * HD)
                .transpose(1, 0, 2)),
            "wk": np.ascontiguousarray(
                wk[:, c * HD : (c + 1) * HD].reshape(KT, 128, HD)
                .transpose(1, 0, 2)),
            "wv": np.ascontiguousarray(
                wv[:, c * HD : (c + 1) * HD].reshape(KT, 128, HD)
                .transpose(1, 0, 2)),
            "wo": wo_r,
            "ckT": np.ascontiguousarray(
                cache_k[:, c][:, perm, :].transpose(0, 2, 1)),
            "cv": np.ascontiguousarray(np.roll(cache_v[:, c], -pos, axis=1)),
            "cosq": cosq, "sinq": sinq, "cosk": cosk, "sink": sink,
            "id128": id128,
        })
    return in_maps


def kernel(x, freqs_cos, freqs_sin, mask, cache_k, cache_v, prefill,
           input_indexes, cache_indexes, wq, wk, wv, wo):
    x = np.asarray(x, np.float32)
    freqs_cos = np.asarray(freqs_cos, np.float32)
    freqs_sin = np.asarray(freqs_sin, np.float32)
    mask = np.asarray(mask, np.float32)
    cache_k = np.asarray(cache_k, np.float32)
    cache_v = np.asarray(cache_v, np.float32)
    wq = np.asarray(wq, np.float32)
    wk = np.asarray(wk, np.float32)
    wv = np.asarray(wv, np.float32)
    wo = np.asarray(wo, np.float32)

    pre = np.asarray(prefill)
    if int(pre.reshape(-1)[0]) if pre.size else 0:
        return _reference_fallback(x, freqs_cos, freqs_sin, mask, cache_k, cache_v,
                                   1, input_indexes, cache_indexes, wq, wk, wv, wo)

    if np.any(mask):
        return _reference_fallback(x, freqs_cos, freqs_sin, mask, cache_k, cache_v,
                                   0, input_indexes, cache_indexes, wq, wk, wv, wo)

    pos = int(np.asarray(input_indexes).reshape(-1)[0])

    from concourse.bass_utils import run_bass_kernel_spmd

    nc = _get_nc()
    in_maps = make_in_maps(x, freqs_cos, freqs_sin, mask, cache_k, cache_v, pos,
                           wq, wk, wv, wo)

    res = run_bass_kernel_spmd(nc, in_maps, list(range(NCORES))).results

    out = np.zeros((B, D), np.float32)
    knew = np.empty((B, KVH, HD), np.float32)
    vnew = np.empty((B, KVH, HD), np.float32)
    for c in range(NCORES):
        out[0:4] += res[c]["out0"]
        out[4:8] += res[c]["out1"]
        knew[:, c] = res[c]["knew"]
        vnew[:, c] = res[c]["vnew"]

    ck_out = cache_k.copy()
    cv_out = cache_v.copy()
    ck_out[:, :, pos, :] = knew
    cv_out[:, :, pos, :] = vnew
    return out.reshape(B, S, H * HD), ck_out, cv_out
